# revision 15
# baseline (speedup 1.0000x reference)
"""Trainium2 Bass kernel for nn_DepthTransformerLayer.

Strategy:
  - Pure data parallelism: B=8192 sharded as 1024 rows per NeuronCore (8 cores),
    weights replicated. No collectives.
  - Only the LAST position of each depth k survives to the output, so:
      * q is computed only for position k (causal row k sees all of 0..k -> no mask)
      * the FFN runs on 1 position per depth
      * K/V are computed for the full prefix 0..k
  - Feature-major on-chip layout [d on partitions, batch in free]: all matmuls
    chain without transposes. Host pre-transposes x and the output back.
  - bf16 for big matmuls (fp32 PSUM accumulation), f32r (tf32-like) for the
    small "indicator" matmuls that do per-head cross-partition reductions and
    partition broadcasts, fp32 for everything elementwise.
  - rsqrt computed as exp(-0.5*ln(v/D + eps)); silu as G*Sigmoid(G).
  - Host prep: fold g_attn/g_ffn and HD^-0.5 into the weights, cast to bf16.

Self-contained: hardcodes all shapes; no sibling imports.
"""
import numpy as np
import ml_dtypes
from contextlib import ExitStack

import concourse.bass as bass
import concourse.mybir as mybir
import concourse.tile as tile
import concourse.tile_utils as tile_utils
from concourse.bass_utils import run_bass_kernel_spmd
from concourse.vector_clock import ScopedClock

F32 = mybir.dt.float32
BF16 = mybir.dt.bfloat16
F32R = mybir.dt.float32r
AF = mybir.ActivationFunctionType

NCORES = 8
B, KDEPTH, D, MLP, H = 8192, 8, 512, 2048, 8
HD = D // H
EPS = 1e-6
B_LOC = B // NCORES          # 1024 batch rows per core
BN = 512                     # batch columns processed per pass
NPASS = B_LOC // BN          # 2
import os as _os
_KLIM = int(_os.environ.get("KLIM", str(KDEPTH)))
_PLIM = int(_os.environ.get("PLIM", str(NPASS)))
_SKIP_ATTN = _os.environ.get("SKIP_ATTN", "0") == "1"
_SKIP_FFN = _os.environ.get("SKIP_FFN", "0") == "1"
ND = D // 128                # 4 partition tiles over d
NM = MLP // 128              # 16 partition tiles over mlp
P = 128

# raise the stale allocator cap (224KB phys / 208 usable on trn2)
tile_utils.max_sbuf_usage = 206 * 1024


def _patched_drain_and_barrier(self, tick_clock, wait_clock):
    # This walrus build rejects >1 sem-wait on a single Drain ("Too many sync
    # wait commands"); spread the tile-tail waits across single-wait drains.
    nc = self.nc
    drain_inst = nc.sync.drain()
    wait_clock.add_sem_waits(drain_inst.ins, ScopedClock({None: tick_clock.global_clock}))
    si = drain_inst.ins.sync_info
    waits = list(si.on_wait or []) if si is not None else []
    if len(waits) > 1:
        drain_inst.ins.sync_info = mybir.SyncInfo(
            on_wait=[waits[0]], on_update=list(si.on_update or [])
        )
        for w in waits[1:]:
            d = nc.sync.drain()
            d.ins.sync_info = mybir.SyncInfo(on_wait=[w], on_update=[])
    nc.all_engine_barrier()
    assert self.sems is not None
    popped = nc._tile_sem_poison_stack.pop()
    assert popped is self._sem_poison
    nc.clear_and_free_semaphores(list(self.sems.allocated().values()))
    nc.all_engine_barrier()


tile.TileContext._drain_and_barrier = _patched_drain_and_barrier


def build_nc():
    nc = bass.Bass(trn_type="TRN2")
    xT = nc.dram_tensor("xT", [D, KDEPTH, B_LOC], F32, kind="ExternalInput")
    wq = nc.dram_tensor("wq", [KDEPTH, D, D], BF16, kind="ExternalInput")
    wk = nc.dram_tensor("wk", [KDEPTH, D, D], BF16, kind="ExternalInput")
    wv = nc.dram_tensor("wv", [KDEPTH, D, D], BF16, kind="ExternalInput")
    wo = nc.dram_tensor("wo", [KDEPTH, D, D], BF16, kind="ExternalInput")
    wg = nc.dram_tensor("wg", [KDEPTH, D, MLP], BF16, kind="ExternalInput")
    wu = nc.dram_tensor("wu", [KDEPTH, D, MLP], BF16, kind="ExternalInput")
    wd = nc.dram_tensor("wd", [KDEPTH, MLP, D], BF16, kind="ExternalInput")
    hsel = nc.dram_tensor("hsel", [ND, P, H], F32, kind="ExternalInput")
    hexp = nc.dram_tensor("hexp", [ND, H, P], BF16, kind="ExternalInput")
    ones_col = nc.dram_tensor("ones_col", [P, 1], F32, kind="ExternalInput")
    ones_row = nc.dram_tensor("ones_row", [1, P], F32, kind="ExternalInput")
    outT = nc.dram_tensor("outT", [D, KDEPTH, B_LOC], F32, kind="ExternalOutput")

    with tile.TileContext(nc) as tc:
        with ExitStack() as ctx:
            _body(ctx, tc, nc, xT, wq, wk, wv, wo, wg, wu, wd,
                  hsel, hexp, ones_col, ones_row, outT)
    _split_multiwaits(nc)
    return nc


def _split_multiwaits(nc, maxw=1):
    """This walrus build rejects >1 sem-wait on several instruction structs
    ("Too many sync wait commands"). Hoist extra waits onto single-wait NOPs
    inserted just before the owning instruction on the same engine."""
    n_split = 0
    for f in nc.m.functions:
        for bb in f.blocks:
            il = bb.instructions
            i = 0
            while i < len(il):
                inst = il[i]
                si = inst.sync_info
                waits = list(si.on_wait or []) if si is not None else []
                if len(waits) > maxw:
                    extras = waits[:-maxw]
                    inst.sync_info = mybir.SyncInfo(
                        on_wait=waits[-maxw:], on_update=list(si.on_update or []))
                    for w_i, w in enumerate(extras):
                        nop = mybir.InstEventSemaphore(
                            name=f"{inst.name}_hw{w_i}",
                            engine=inst.engine,
                            sync_info=mybir.SyncInfo(on_wait=[w], on_update=[]),
                        )
                        il.insert(i, nop)
                        i += 1
                    n_split += 1
                i += 1
    return n_split


def _body(ctx, tc, nc, xT, wq, wk, wv, wo, wg, wu, wd, hsel, hexp,
          ones_col, ones_row, outT):
    consts = ctx.enter_context(tc.tile_pool(name="consts", bufs=1))
    xpool = ctx.enter_context(tc.tile_pool(name="xp", bufs=2))     # x prep chunks
    xhatp = ctx.enter_context(tc.tile_pool(name="xhat", bufs=1))   # resident bf16
    wap = ctx.enter_context(tc.tile_pool(name="wap", bufs=2))      # attn weights
    wfp = ctx.enter_context(tc.tile_pool(name="wfp", bufs=6))      # wg/wu half-tiles
    wdp = ctx.enter_context(tc.tile_pool(name="wdp", bufs=6))      # wd k-tiles
    big = ctx.enter_context(tc.tile_pool(name="big", bufs=1))      # q/oacc/x1/osc/n2
    scr = ctx.enter_context(tc.tile_pool(name="scr", bufs=5))      # [P,512] scratch
    small = ctx.enter_context(tc.tile_pool(name="small", bufs=2))  # [8,*] and misc
    small1 = ctx.enter_context(tc.tile_pool(name="small1", bufs=1))
    hp = ctx.enter_context(tc.tile_pool(name="hp", bufs=4))        # ffn hidden tiles
    outp = ctx.enter_context(tc.tile_pool(name="outp", bufs=1))
    ps = ctx.enter_context(tc.tile_pool(name="ps", bufs=8, space="PSUM"))

    # --- constants ---
    hsel_sb = consts.tile([P, ND, H], F32R)
    nc.sync.dma_start(out=hsel_sb, in_=hsel.rearrange("t p h -> p t h").bitcast(F32R))
    hexp_sb = consts.tile([H, ND, P], BF16)
    nc.sync.dma_start(out=hexp_sb, in_=hexp.rearrange("t h p -> h t p"))
    onec_sb = consts.tile([P, 1], F32R)
    nc.sync.dma_start(out=onec_sb, in_=ones_col[:, :].bitcast(F32R))
    oner_sb = consts.tile([1, P], F32R)
    nc.sync.dma_start(out=oner_sb, in_=ones_row[:, :].bitcast(F32R))
    eps_sb = consts.tile([P, 1], F32)
    nc.vector.memset(eps_sb, EPS)

    for p_i in range(_PLIM):
        br = p_i * BN  # batch offset of this pass

        # ---------- xhat = x * rsqrt(mean(x^2) + eps) -> bf16, feature-major --
        xhat = [xhatp.tile([P, KDEPTH, BN], BF16, tag=f"xh{t}", name=f"xh{t}")
                for t in range(ND)]
        for pos in range(KDEPTH):
            x_c = [xpool.tile([P, BN], F32, tag=f"x{t}", name=f"x{t}") for t in range(ND)]
            for t in range(ND):
                nc.gpsimd.dma_start(out=x_c[t],
                                    in_=xT[t * P:(t + 1) * P, pos, br:br + BN])
            vs_ps = ps.tile([1, BN], F32, tag="ps")
            for t in range(ND):
                xsq = scr.tile([P, BN], F32R, tag="scr")
                nc.scalar.activation(xsq, x_c[t], AF.Square)
                nc.tensor.matmul(vs_ps, onec_sb, xsq,
                                 start=(t == 0), stop=(t == ND - 1))
            vs_sb = small.tile([1, BN], F32R, tag="vs")
            nc.scalar.activation(vs_sb, vs_ps, AF.Copy)
            ve_ps = ps.tile([P, BN], F32, tag="ps")
            nc.tensor.matmul(ve_ps, oner_sb, vs_sb,
                             start=True, stop=True)
            lnv = scr.tile([P, BN], F32, tag="scr")
            nc.scalar.activation(lnv, ve_ps, AF.Ln, bias=eps_sb, scale=1.0 / D)
            r1 = scr.tile([P, BN], F32, tag="scr")
            nc.scalar.activation(r1, lnv, AF.Exp, scale=-0.5)
            for t in range(ND):
                nc.vector.tensor_mul(xhat[t][:, pos, :], x_c[t], r1)

        # ---------- depth loop ----------
        for k in range(_KLIM):
            L = k + 1
            # attn weights for this depth (one strided DMA per matrix)
            wq_sb = wap.tile([P, ND, D], BF16, tag="wq")
            wk_sb = wap.tile([P, ND, D], BF16, tag="wk")
            wv_sb = wap.tile([P, ND, D], BF16, tag="wv")
            wo_sb = wap.tile([P, ND, D], BF16, tag="wo")
            for wsb, wdr in ((wq_sb, wq), (wk_sb, wk), (wv_sb, wv), (wo_sb, wo)):
                nc.sync.dma_start(out=wsb,
                                  in_=wdr[k].rearrange("(t p) m -> p t m", p=P))

            # q = xhat[:, k] @ Wq'  -> q_sb fp32 [P, ND, BN]
            q_sb = big.tile([P, ND, BN], F32, tag="q")
            for m in range(ND):
                q_ps = ps.tile([P, BN], F32, tag="ps")
                for kt in range(ND):
                    nc.tensor.matmul(q_ps, wq_sb[:, kt, m * P:(m + 1) * P],
                                     xhat[kt][:, k, :],
                                     start=(kt == 0), stop=(kt == ND - 1))
                nc.scalar.activation(q_sb[:, m, :], q_ps, AF.Copy)

            e_sb = small1.tile([H, KDEPTH, BN], BF16, tag="e")
            o_acc = big.tile([P, ND, BN], F32, tag="oacc")
            for j in range(L):
                # K_j (psum) + score mul + per-head reduce -> scores psum [H, BN]
                s_ps = ps.tile([H, BN], F32, tag="ps")
                for t in range(ND):
                    kj_ps = ps.tile([P, BN], F32, tag="ps")
                    for kt in range(ND):
                        nc.tensor.matmul(kj_ps, wk_sb[:, kt, t * P:(t + 1) * P],
                                         xhat[kt][:, j, :],
                                         start=(kt == 0), stop=(kt == ND - 1))
                    mul = scr.tile([P, BN], F32R, tag="scr")
                    nc.vector.tensor_mul(mul, kj_ps, q_sb[:, t, :])
                    nc.tensor.matmul(s_ps, hsel_sb[:, t, :], mul,
                                     start=(t == 0), stop=(t == ND - 1))
                nc.scalar.activation(e_sb[:, j, :], s_ps, AF.Exp)
                # V_j (psum) ; e_exp = expand(e_j) ; o_acc += e_exp * V_j
                for t in range(ND):
                    vj_ps = ps.tile([P, BN], F32, tag="ps")
                    for kt in range(ND):
                        nc.tensor.matmul(vj_ps, wv_sb[:, kt, t * P:(t + 1) * P],
                                         xhat[kt][:, j, :],
                                         start=(kt == 0), stop=(kt == ND - 1))
                    ee_ps = ps.tile([P, BN], F32, tag="ps")
                    nc.tensor.matmul(ee_ps, hexp_sb[:, t, :], e_sb[:, j, :],
                                     start=True, stop=True)
                    ee_sb = small.tile([P, BN], BF16, tag="ee")
                    nc.scalar.activation(ee_sb, ee_ps, AF.Copy)
                    if j == 0:
                        nc.vector.tensor_mul(o_acc[:, t, :], vj_ps, ee_sb)
                    else:
                        tmp = small.tile([P, BN], BF16, tag="otmp")
                        nc.vector.tensor_mul(tmp, vj_ps, ee_sb)
                        nc.vector.tensor_add(o_acc[:, t, :], o_acc[:, t, :], tmp)

            # softmax denominator & rescale
            esum = small1.tile([H, BN], F32, tag="esum")
            nc.vector.tensor_reduce(esum,
                                    e_sb[:, 0:L, :].rearrange("h l b -> h b l"),
                                    axis=mybir.AxisListType.X,
                                    op=mybir.AluOpType.add)
            recip = small1.tile([H, BN], F32, tag="recip")
            nc.vector.reciprocal(recip, esum)
            recip_bf = small.tile([H, BN], BF16, tag="recipbf")
            nc.vector.tensor_copy(recip_bf, recip)
            osc = big.tile([P, ND, BN], BF16, tag="osc")
            for t in range(ND):
                re_ps = ps.tile([P, BN], F32, tag="ps")
                nc.tensor.matmul(re_ps, hexp_sb[:, t, :], recip_bf,
                                 start=True, stop=True)
                nc.vector.tensor_mul(osc[:, t, :], re_ps, o_acc[:, t, :])

            # attn = osc @ Wo ; x1 = x_k + attn
            xk_sb = outp.tile([P, ND, BN], F32, tag="xk")
            nc.gpsimd.dma_start(
                out=xk_sb,
                in_=xT[:, k, br:br + BN].rearrange("(t p) b -> p t b", p=P))
            x1 = big.tile([P, ND, BN], F32, tag="x1")
            for m in range(ND):
                at_ps = ps.tile([P, BN], F32, tag="ps")
                for kt in range(ND):
                    nc.tensor.matmul(at_ps, wo_sb[:, kt, m * P:(m + 1) * P],
                                     osc[:, kt, :],
                                     start=(kt == 0), stop=(kt == ND - 1))
                nc.vector.tensor_add(x1[:, m, :], at_ps, xk_sb[:, m, :])

            # n2 = x1 * rsqrt(mean(x1^2)+eps) -> bf16
            vs2_ps = ps.tile([1, BN], F32, tag="ps")
            for t in range(ND):
                xsq2 = scr.tile([P, BN], F32R, tag="scr")
                nc.scalar.activation(xsq2, x1[:, t, :], AF.Square)
                nc.tensor.matmul(vs2_ps, onec_sb, xsq2,
                                 start=(t == 0), stop=(t == ND - 1))
            vs2_sb = small.tile([1, BN], F32R, tag="vs")
            nc.scalar.activation(vs2_sb, vs2_ps, AF.Copy)
            ve2_ps = ps.tile([P, BN], F32, tag="ps")
            nc.tensor.matmul(ve2_ps, oner_sb, vs2_sb,
                             start=True, stop=True)
            lnv2 = scr.tile([P, BN], F32, tag="scr")
            nc.scalar.activation(lnv2, ve2_ps, AF.Ln, bias=eps_sb, scale=1.0 / D)
            r2 = scr.tile([P, BN], F32, tag="scr")
            nc.scalar.activation(r2, lnv2, AF.Exp, scale=-0.5)
            n2 = big.tile([P, ND, BN], BF16, tag="n2")
            for t in range(ND):
                nc.vector.tensor_mul(n2[:, t, :], x1[:, t, :], r2)

            # FFN: h = G*sigmoid(G)*U with G = n2@Wg, U = n2@Wu; out = x1 + h@Wd
            MH = MLP // 2
            wg_sb = {}
            wu_sb = {}
            for half in range(2):
                for kt in range(ND):
                    gt = wfp.tile([P, MH], BF16, tag="wg", name="wg_sb")
                    nc.sync.dma_start(
                        out=gt, in_=wg[k, kt * P:(kt + 1) * P,
                                       half * MH:(half + 1) * MH])
                    wg_sb[(half, kt)] = gt
                    ut = wfp.tile([P, MH], BF16, tag="wu", name="wu_sb")
                    nc.sync.dma_start(
                        out=ut, in_=wu[k, kt * P:(kt + 1) * P,
                                       half * MH:(half + 1) * MH])
                    wu_sb[(half, kt)] = ut
            out_sb = outp.tile([P, ND, BN], F32, tag="out")
            f_ps = [ps.tile([P, BN], F32, tag="ps", name=f"f_ps{m}")
                    for m in range(ND)]
            for mt in range(NM):
                half, mh = mt // 8, (mt % 8) * P
                g_ps = ps.tile([P, BN], F32, tag="ps")
                for kt in range(ND):
                    nc.tensor.matmul(g_ps, wg_sb[(half, kt)][:, mh:mh + P],
                                     n2[:, kt, :],
                                     start=(kt == 0), stop=(kt == ND - 1))
                sig = small.tile([P, BN], BF16, tag="sg")
                nc.scalar.activation(sig, g_ps, AF.Sigmoid)
                silu = small.tile([P, BN], BF16, tag="silu")
                nc.vector.tensor_mul(silu, g_ps, sig)
                u_ps = ps.tile([P, BN], F32, tag="ps")
                for kt in range(ND):
                    nc.tensor.matmul(u_ps, wu_sb[(half, kt)][:, mh:mh + P],
                                     n2[:, kt, :],
                                     start=(kt == 0), stop=(kt == ND - 1))
                h_t = hp.tile([P, BN], BF16, tag="h")
                nc.vector.tensor_mul(h_t, u_ps, silu)
                wd_t = wdp.tile([P, D], BF16, tag="wd", name="wd_t")
                nc.sync.dma_start(out=wd_t, in_=wd[k, mt * P:(mt + 1) * P, :])
                for m in range(ND):
                    nc.tensor.matmul(f_ps[m], wd_t[:, m * P:(m + 1) * P], h_t,
                                     start=(mt == 0), stop=(mt == NM - 1))
            for m in range(ND):
                nc.vector.tensor_add(out_sb[:, m, :], f_ps[m], x1[:, m, :])
            nc.gpsimd.dma_start(
                out=outT[:, k, br:br + BN].rearrange("(t p) b -> p t b", p=P),
                in_=out_sb)


def _host_prep(x, Wq, Wk, Wv, Wo, Wg, Wu, Wd, g_attn, g_ffn):
    bf = ml_dtypes.bfloat16
    scale = HD ** -0.5
    wq_h = (np.asarray(g_attn)[:, :, None] * np.asarray(Wq) * scale).astype(bf)
    wk_h = (np.asarray(g_attn)[:, :, None] * np.asarray(Wk)).astype(bf)
    wv_h = (np.asarray(g_attn)[:, :, None] * np.asarray(Wv)).astype(bf)
    wo_h = np.asarray(Wo).astype(bf)
    wg_h = (np.asarray(g_ffn)[:, :, None] * np.asarray(Wg)).astype(bf)
    wu_h = (np.asarray(g_ffn)[:, :, None] * np.asarray(Wu)).astype(bf)
    wd_h = np.asarray(Wd).astype(bf)
    xT = np.ascontiguousarray(np.asarray(x, np.float32).transpose(2, 1, 0))
    hsel = np.zeros((ND, P, H), np.float32)
    hexp = np.zeros((ND, H, P), np.float32)
    for t in range(ND):
        for p in range(P):
            hh = (t * P + p) // HD
            hsel[t, p, hh] = 1.0
            hexp[t, hh, p] = 1.0
    return dict(
        xT=xT, wq=wq_h, wk=wk_h, wv=wv_h, wo=wo_h, wg=wg_h, wu=wu_h, wd=wd_h,
        hsel=hsel, hexp=hexp.astype(bf),
        ones_col=np.ones((P, 1), np.float32),
        ones_row=np.ones((1, P), np.float32),
    )


_NC_CACHE = None


def kernel(x, causal_mask, Wq, Wk, Wv, Wo, Wg, Wu, Wd, g_attn, g_ffn,
           active_depth, _trace=False):
    global _NC_CACHE
    assert int(active_depth) == KDEPTH, "kernel hardcodes active_depth=8"
    full = _host_prep(x, Wq, Wk, Wv, Wo, Wg, Wu, Wd, g_attn, g_ffn)
    in_maps = []
    for c in range(NCORES):
        m = dict(full)
        m["xT"] = np.ascontiguousarray(full["xT"][:, :, c * B_LOC:(c + 1) * B_LOC])
        in_maps.append(m)
    if _NC_CACHE is None:
        _NC_CACHE = build_nc()
    nc = _NC_CACHE
    res = run_bass_kernel_spmd(nc, in_maps, core_ids=list(range(NCORES)),
                               trace=_trace)
    outs = [np.asarray(r["outT"]).transpose(2, 1, 0) for r in res.results]
    out = np.concatenate(outs, axis=0).astype(np.float32)
    if _trace:
        return out, res
    return out
